# revision 12
# baseline (speedup 1.0000x reference)
# GQA attention block on 8 Trainium2 NeuronCores.
# Sharding: core = (batch b in {0,1}) x (tensor-parallel t in {0..3}).
# Each core: batch row b, 4 query heads {4t..4t+3}, 2 kv heads {2t, 2t+1}.
# W_Q/W_K/W_V split column-wise (per-head), W_O row-wise; the 4 TP partial
# outputs per batch are summed on the host (the "all-reduce").
#
# Schedule: K-pair projection paced against the xT DMA stream, then V,
# then Q0/Q1; attention h0/h1 runs with Q2/Q3 projection matmuls woven
# into the exp-latency slots; attention h2/h3 runs with the output
# projection woven in the same way.  Softmax denominators are computed
# by DVE running-adds of the exp tiles plus a single ones-row matmul
# per block (instead of a full second pass of PE matmuls).
import math
import sys

sys.path.insert(0, "/opt/trn_rl_repo")

import ml_dtypes
import numpy as np

import concourse.bacc as bacc
import concourse.bass as bass
import concourse.mybir as mybir
import concourse.tile as tile
from contextlib import ExitStack

BF = mybir.dt.bfloat16
F32 = mybir.dt.float32
bfnp = ml_dtypes.bfloat16

EMB = 2048
HEADS = 16
G = 2
HD = 128          # head dim
KV = HEADS // G   # 8 kv heads
B = 2
S = 2048
NCORES = 8
TP = 4
HQ = HEADS // TP       # 4 q heads per core
HKV = KV // TP         # 2 kv heads per core
NE = EMB // 128        # 16 contraction chunks
SC4 = S // 512         # 4 s-chunks of 512
SC16 = S // 128        # 16 s-chunks of 128
SCALE = 1.0 / math.sqrt(float(EMB))

_NC = None


def _build_program(loop_n=None):
    nc = bacc.Bacc("TRN2", target_bir_lowering=False, debug=False)

    xT = nc.dram_tensor("xT", (EMB, S), BF, kind="ExternalInput")
    wq = nc.dram_tensor("wq", (EMB, HQ * HD), BF, kind="ExternalInput")
    wk = nc.dram_tensor("wk", (EMB, HKV * HD), BF, kind="ExternalInput")
    wv = nc.dram_tensor("wv", (EMB, HKV * HD), BF, kind="ExternalInput")
    wo = nc.dram_tensor("wo", (HQ * HD, EMB), BF, kind="ExternalInput")
    cosT = nc.dram_tensor("cosT", (HD, S), BF, kind="ExternalInput")
    sinT = nc.dram_tensor("sinT", (HD, S), BF, kind="ExternalInput")
    out = nc.dram_tensor("out", (S, EMB), BF, kind="ExternalOutput")

    with tile.TileContext(nc) as tc, ExitStack() as ctx:
        persist = ctx.enter_context(tc.tile_pool(name="persist", bufs=1))
        # qk_sb j-blocks: 0..3 = roped Q heads, 4..5 = roped K kv-heads; [d, s]
        qk_sb = persist.tile([128, HQ + HKV, S], BF)
        # V in [t, d] layout: [t_part, t_chunk, kvl*128+d]
        v_sb = persist.tile([128, SC16, HKV * HD], BF)
        ctx_sb = persist.tile([128, HQ, S], BF)      # [d, head, s]
        wo_sb = persist.tile([128, HQ, EMB], BF)     # [d, head, e_out]
        cos_sb = persist.tile([128, S], BF)
        sin_sb = persist.tile([128, S], BF)
        ones_sb = persist.tile([128, 1], BF)
        xt_sb = persist.tile([128, NE, S], BF)
        wq_sb = persist.tile([128, NE, HQ * HD], BF)
        wk_sb = persist.tile([128, NE, HKV * HD], BF)
        wv_sb = persist.tile([128, NE, HKV * HD], BF)
        nc.vector.memset(ones_sb, 1.0)

        rt = ctx.enter_context(tc.tile_pool(name="ropet", bufs=8))

        # RoPE is split into a copy stage (PSUM read + half-swap DMA start)
        # and a mul stage (cos/sin muls + add).  Emitting all copies before
        # any muls keeps the in-order DVE stream from blocking on the swap
        # DMA round-trip, and releases the projection PSUM banks early.
        def rope_copy(pts, jb, sc):
            xs = rt.tile([128, 512], BF, tag="xs")
            nc.vector.tensor_copy(xs, pts)
            xw = rt.tile([128, 512], BF, tag="xw")
            nc.sync.dma_start(out=xw[0:64, :], in_=xs[64:128, :])
            nc.sync.dma_start(out=xw[64:128, :], in_=xs[0:64, :])
            return (xs, xw, jb, sc)

        def rope_mul(st):
            xs, xw, jb, sc = st
            sl = slice(sc * 512, (sc + 1) * 512)
            nc.vector.tensor_mul(xs, xs, cos_sb[:, sl])
            nc.vector.tensor_mul(xw, xw, sin_sb[:, sl])
            nc.vector.tensor_add(qk_sb[:, jb, sl], xs, xw)

        def _phases():
            # ---------------- input DMAs, in dependency-priority order ----
            # First chunk split fine so the first matmul starts ~1us in.
            nc.sync.dma_start(out=wk_sb[:, 0, :], in_=wk[0:128, :])
            for sc in range(SC4):
                nc.sync.dma_start(
                    out=xt_sb[:, 0, sc * 512:(sc + 1) * 512],
                    in_=xT[0:128, sc * 512:(sc + 1) * 512],
                )
            for c in range(1, NE):
                nc.sync.dma_start(out=xt_sb[:, c, :], in_=xT[c * 128:(c + 1) * 128, :])
                nc.sync.dma_start(out=wk_sb[:, c, :], in_=wk[c * 128:(c + 1) * 128, :])
            nc.sync.dma_start(out=cos_sb, in_=cosT[:, :])
            nc.sync.dma_start(out=sin_sb, in_=sinT[:, :])
            for c in range(NE):
                nc.sync.dma_start(out=wq_sb[:, c, :], in_=wq[c * 128:(c + 1) * 128, :])
            for c in range(NE):
                nc.sync.dma_start(out=wv_sb[:, c, :], in_=wv[c * 128:(c + 1) * 128, :])
            for jb in range(HQ):
                nc.sync.dma_start(out=wo_sb[:, jb, :], in_=wo[jb * 128:(jb + 1) * 128, :])

            # ---------------- A: K-pair projection (DMA-paced) ------------
            with tc.tile_pool(name="ppA", bufs=8, space=bass.MemorySpace.PSUM) as ppA:
                ptsA = {}
                for kvl in range(HKV):
                    for sc in range(SC4):
                        ptsA[(kvl, sc)] = ppA.tile(
                            [128, 512], F32, tag="pts", name=f"ptsA_{kvl}_{sc}"
                        )
                for c in range(NE):
                    for kvl in range(HKV):
                        lhsT = wk_sb[:, c, kvl * 128:(kvl + 1) * 128]
                        for sc in range(SC4):
                            nc.tensor.matmul(
                                ptsA[(kvl, sc)], lhsT,
                                xt_sb[:, c, sc * 512:(sc + 1) * 512],
                                start=(c == 0), stop=(c == NE - 1),
                            )
                stages = [
                    rope_copy(ptsA[(kvl, sc)], HQ + kvl, sc)
                    for kvl in range(HKV) for sc in range(SC4)
                ]
                for st in stages:
                    rope_mul(st)

            # ---------------- B: V projection ------------------------------
            with tc.tile_pool(name="ppV", bufs=4, space=bass.MemorySpace.PSUM) as ppV:
                for st in range(SC16):
                    pv = ppV.tile([128, HKV * HD], F32, tag="pv")
                    for c in range(NE):
                        nc.tensor.matmul(
                            pv,
                            xt_sb[:, c, st * 128:(st + 1) * 128],
                            wv_sb[:, c, :],
                            start=(c == 0), stop=(c == NE - 1),
                        )
                    nc.vector.tensor_copy(v_sb[:, st, :], pv)

            # ---------------- C: Q0/Q1 projection (tile-major) -------------
            # Tile-major order lets each block's rope overlap the next
            # block's accumulation, so attention h0 can start right after
            # the last matmul instead of waiting on a batch of ropes.
            with tc.tile_pool(name="ppC", bufs=4, space=bass.MemorySpace.PSUM) as ppC:
                prev = None
                for q in range(2):
                    for sc in range(SC4):
                        pts = ppC.tile([128, 512], F32, tag="pts",
                                       name=f"ptsC_{q}_{sc}")
                        for c in range(NE):
                            nc.tensor.matmul(
                                pts, wq_sb[:, c, q * 128:(q + 1) * 128],
                                xt_sb[:, c, sc * 512:(sc + 1) * 512],
                                start=(c == 0), stop=(c == NE - 1),
                            )
                        st = rope_copy(pts, q, sc)
                        if prev is not None:
                            rope_mul(prev)
                        prev = st
                rope_mul(prev)

            # ---------------- attention + woven work -----------------------
            exp_f = mybir.ActivationFunctionType.Exp

            with tc.tile_pool(name="psc", bufs=2, space=bass.MemorySpace.PSUM) as psc, \
                 tc.tile_pool(name="pcx", bufs=2, space=bass.MemorySpace.PSUM) as pcx, \
                 tc.tile_pool(name="pdn", bufs=1, space=bass.MemorySpace.PSUM) as pdn, \
                 tc.tile_pool(name="expp", bufs=6) as expp, \
                 tc.tile_pool(name="accp", bufs=2) as accp, \
                 tc.tile_pool(name="misc", bufs=2) as misc:

                def attention_block(h, sc, weave):
                    kvjb = HQ + h // 2
                    kvl = h // 2
                    ssl = slice(sc * 512, (sc + 1) * 512)
                    cps = pcx.tile([128, 512], F32, tag="cps")
                    acc = accp.tile([128, 512], BF, tag="acc")
                    for t in range(SC16):
                        sps = psc.tile([128, 512], F32, tag="sps")
                        nc.tensor.matmul(
                            sps,
                            qk_sb[:, kvjb, t * 128:(t + 1) * 128],
                            qk_sb[:, h, ssl],
                            start=True, stop=True,
                        )
                        ex = expp.tile([128, 512], BF, tag="ex")
                        nc.scalar.activation(ex, sps, exp_f, scale=SCALE)
                        nc.tensor.matmul(
                            cps,
                            v_sb[:, t, kvl * 128:(kvl + 1) * 128],
                            ex,
                            start=(t == 0), stop=(t == SC16 - 1),
                        )
                        if t == 0:
                            nc.vector.tensor_copy(acc, ex)
                        else:
                            nc.vector.tensor_add(acc, acc, ex)
                        if t % 2 == 1:
                            u = next(weave, None)
                            if u is not None:
                                u()
                    dps = pdn.tile([1, 512], F32, tag="dps")
                    nc.tensor.matmul(dps, ones_sb, acc, start=True, stop=True)
                    rc = misc.tile([1, 512], F32, tag="rc")
                    nc.vector.reciprocal(rc, dps)
                    rb = misc.tile([128, 512], F32, tag="rb")
                    nc.gpsimd.partition_broadcast(rb, rc)
                    nc.vector.tensor_mul(ctx_sb[:, h, ssl], cps, rb)

                # ---- D: h0/h1 with Q2/Q3 projection woven in ----
                with tc.tile_pool(name="pp2", bufs=2, space=bass.MemorySpace.PSUM) as pp2:

                    def qproj_units():
                        pending = []  # deferred rope-mul closures
                        for q in (2, 3):
                            for scpair in (0, 1):
                                scs = (2 * scpair, 2 * scpair + 1)
                                pts = {
                                    sc: pp2.tile(
                                        [128, 512], F32, tag="pts2",
                                        name=f"pts2_{q}_{sc}",
                                    )
                                    for sc in scs
                                }
                                for c in range(NE):
                                    def unit(q=q, scs=scs, c=c, pts=pts):
                                        lhsT = wq_sb[:, c, q * 128:(q + 1) * 128]
                                        for sc in scs:
                                            nc.tensor.matmul(
                                                pts[sc], lhsT,
                                                xt_sb[:, c, sc * 512:(sc + 1) * 512],
                                                start=(c == 0), stop=(c == NE - 1),
                                            )
                                    yield unit
                                    if c == 3 and pending:
                                        yield pending.pop(0)

                                def copy_unit(q=q, scs=scs, pts=pts):
                                    sts = [rope_copy(pts[sc], q, sc) for sc in scs]

                                    def mul_unit(sts=sts):
                                        for st in sts:
                                            rope_mul(st)
                                    pending.append(mul_unit)
                                yield copy_unit
                        while pending:
                            yield pending.pop(0)

                    weave = qproj_units()
                    for h in (0, 1):
                        for sc in range(SC4):
                            attention_block(h, sc, weave)
                    for u in weave:
                        u()

                # ---- E: h2/h3 with O-projection woven in ----
                with tc.tile_pool(name="pou", bufs=3, space=bass.MemorySpace.PSUM) as pou, \
                     tc.tile_pool(name="outp", bufs=3) as outp:

                    def oproj_unit(so, ec):
                        ops = pou.tile([128, 512], F32, tag="ops")
                        for hl in range(HQ):
                            nc.tensor.matmul(
                                ops,
                                ctx_sb[:, hl, so * 128:(so + 1) * 128],
                                wo_sb[:, hl, ec * 512:(ec + 1) * 512],
                                start=(hl == 0), stop=(hl == HQ - 1),
                            )
                        ot = outp.tile([128, 512], BF, tag="ot")
                        nc.any.tensor_copy(ot, ops)
                        nc.sync.dma_start(
                            out=out[so * 128:(so + 1) * 128, ec * 512:(ec + 1) * 512],
                            in_=ot,
                        )

                    ready = []  # (so, ec) units whose inputs are complete

                    def drain(weave_iter):
                        for u in weave_iter:
                            u()

                    def make_weave():
                        while True:
                            if ready:
                                so, ec = ready.pop(0)
                                yield (lambda so=so, ec=ec: oproj_unit(so, ec))
                            else:
                                yield None

                    weave = make_weave()
                    for sc in range(SC4):
                        attention_block(2, sc, weave)
                        attention_block(3, sc, weave)
                        for so in range(4 * sc, 4 * sc + 4):
                            for ec in range(SC4):
                                ready.append((so, ec))
                    while ready:
                        so, ec = ready.pop(0)
                        oproj_unit(so, ec)

        if loop_n is not None:
            with tc.For_i(0, loop_n, 1):
                _phases()
        else:
            _phases()

    nc.compile()
    return nc


def _get_nc():
    global _NC
    if _NC is None:
        _NC = _build_program()
    return _NC


def _rope_tables():
    half = HD // 2
    inv_freq = 1.0 / (10000.0 ** (np.arange(half, dtype=np.float64) * 2.0 / HD))
    ang = np.arange(S, dtype=np.float64)[:, None] * inv_freq[None, :]  # (S, 64)
    cos = np.concatenate([np.cos(ang), np.cos(ang)], axis=1).T  # (128, S)
    sin = np.concatenate([-np.sin(ang), np.sin(ang)], axis=1).T  # pre-signed
    return (np.ascontiguousarray(cos).astype(bfnp),
            np.ascontiguousarray(sin).astype(bfnp))


def build_in_maps(x, W_Q, W_K, W_V, W_O):
    x = np.asarray(x, dtype=np.float32)
    W_Q = np.asarray(W_Q, dtype=np.float32)
    W_K = np.asarray(W_K, dtype=np.float32)
    W_V = np.asarray(W_V, dtype=np.float32)
    W_O = np.asarray(W_O, dtype=np.float32)
    cos, sin = _rope_tables()
    in_maps = []
    xTb = [np.ascontiguousarray(x[b].T).astype(bfnp) for b in range(B)]
    for b in range(B):
        for t in range(TP):
            qheads = list(range(HQ * t, HQ * t + HQ))
            kvheads = [HKV * t + i for i in range(HKV)]
            idxq = [d * HEADS + h for h in qheads for d in range(HD)]
            idxkv = [d * KV + kv for kv in kvheads for d in range(HD)]
            rows_o = [h * HD + d for h in qheads for d in range(HD)]
            in_maps.append(dict(
                xT=xTb[b],
                wq=np.ascontiguousarray(W_Q[idxq, :].T).astype(bfnp),
                wk=np.ascontiguousarray(W_K[idxkv, :].T).astype(bfnp),
                wv=np.ascontiguousarray(W_V[idxkv, :].T).astype(bfnp),
                wo=np.ascontiguousarray(W_O[:, rows_o].T).astype(bfnp),
                cosT=cos,
                sinT=sin,
            ))
    return in_maps


def combine_outs(outs):
    out = np.empty((B, S, EMB), dtype=np.float32)
    for b in range(B):
        acc = outs[TP * b].astype(np.float32).copy()
        for t in range(1, TP):
            acc += outs[TP * b + t]
        out[b] = acc
    return out


LAST_RESULTS = None


def kernel(x, W_Q, W_K, W_V, W_O):
    global LAST_RESULTS
    from concourse.bass_utils import run_bass_kernel_spmd

    nc = _get_nc()
    in_maps = build_in_maps(x, W_Q, W_K, W_V, W_O)
    res = run_bass_kernel_spmd(nc, in_maps, list(range(NCORES)))
    LAST_RESULTS = res
    outs = [r["out"] for r in res.results]
    return combine_outs(outs)


# revision 16
# speedup vs baseline: 1.1199x; 1.1199x over previous
# GQA attention block on 8 Trainium2 NeuronCores.
# Sharding: core = (batch b in {0,1}) x (tensor-parallel t in {0..3}).
# Each core: batch row b, 4 query heads {4t..4t+3}, 2 kv heads {2t, 2t+1}.
# W_Q/W_K/W_V split column-wise (per-head), W_O row-wise; the 4 TP partial
# outputs per batch are summed on the host (the "all-reduce").
#
# Schedule: K-pair projection paced against the xT DMA stream, then V,
# then Q0/Q1; attention h0/h1 runs with Q2/Q3 projection matmuls woven
# into the exp-latency slots; attention h2/h3 runs with the output
# projection woven in the same way.  Softmax denominators are computed
# by DVE running-adds of the exp tiles plus a single ones-row matmul
# per block (instead of a full second pass of PE matmuls).
import math
import sys

sys.path.insert(0, "/opt/trn_rl_repo")

import ml_dtypes
import numpy as np

import concourse.bacc as bacc
import concourse.bass as bass
import concourse.mybir as mybir
import concourse.tile as tile
from contextlib import ExitStack

BF = mybir.dt.bfloat16
F32 = mybir.dt.float32
bfnp = ml_dtypes.bfloat16

EMB = 2048
HEADS = 16
G = 2
HD = 128          # head dim
KV = HEADS // G   # 8 kv heads
B = 2
S = 2048
NCORES = 8
TP = 4
HQ = HEADS // TP       # 4 q heads per core
HKV = KV // TP         # 2 kv heads per core
NE = EMB // 128        # 16 contraction chunks
SC4 = S // 512         # 4 s-chunks of 512
SC16 = S // 128        # 16 s-chunks of 128
SCALE = 1.0 / math.sqrt(float(EMB))

_NC = None


def _build_program(loop_n=None):
    nc = bacc.Bacc("TRN2", target_bir_lowering=False, debug=False)

    xT = nc.dram_tensor("xT", (EMB, S), BF, kind="ExternalInput")
    wq = nc.dram_tensor("wq", (EMB, HQ * HD), BF, kind="ExternalInput")
    wk = nc.dram_tensor("wk", (EMB, HKV * HD), BF, kind="ExternalInput")
    wv = nc.dram_tensor("wv", (EMB, HKV * HD), BF, kind="ExternalInput")
    wo = nc.dram_tensor("wo", (HQ * HD, EMB), BF, kind="ExternalInput")
    cosT = nc.dram_tensor("cosT", (HD, S), BF, kind="ExternalInput")
    sinT = nc.dram_tensor("sinT", (HD, S), BF, kind="ExternalInput")
    out = nc.dram_tensor("out", (S, EMB), BF, kind="ExternalOutput")

    with tile.TileContext(nc) as tc, ExitStack() as ctx:
        persist = ctx.enter_context(tc.tile_pool(name="persist", bufs=1))
        # qk_sb j-blocks: 0..3 = roped Q heads, 4..5 = roped K kv-heads; [d, s]
        qk_sb = persist.tile([128, HQ + HKV, S], BF)
        # V in [t, d] layout: [t_part, t_chunk, kvl*128+d]
        v_sb = persist.tile([128, SC16, HKV * HD], BF)
        ctx_sb = persist.tile([128, HQ, S], BF)      # [d, head, s]
        wo_sb = persist.tile([128, HQ, EMB], BF)     # [d, head, e_out]
        cos_sb = persist.tile([128, S], BF)
        sin_sb = persist.tile([128, S], BF)
        ones_sb = persist.tile([128, 1], BF)
        xt_sb = persist.tile([128, NE, S], BF)
        wq_sb = persist.tile([128, NE, HQ * HD], BF)
        wk_sb = persist.tile([128, NE, HKV * HD], BF)
        wv_sb = persist.tile([128, NE, HKV * HD], BF)
        nc.vector.memset(ones_sb, 1.0)

        rt = ctx.enter_context(tc.tile_pool(name="ropet", bufs=8))

        # RoPE is split into a copy stage (PSUM read + half-swap DMA start)
        # and a mul stage (cos/sin muls + add).  Emitting all copies before
        # any muls keeps the in-order DVE stream from blocking on the swap
        # DMA round-trip, and releases the projection PSUM banks early.
        def rope_copy(pts, jb, sc, eng="scalar"):
            xs = rt.tile([128, 512], BF, tag="xs")
            if eng == "scalar":
                nc.scalar.copy(xs, pts)
            else:
                nc.vector.tensor_copy(xs, pts)
            xw = rt.tile([128, 512], BF, tag="xw")
            nc.sync.dma_start(out=xw[0:64, :], in_=xs[64:128, :])
            nc.sync.dma_start(out=xw[64:128, :], in_=xs[0:64, :])
            return (xs, xw, jb, sc)

        def rope_mul(st):
            xs, xw, jb, sc = st
            sl = slice(sc * 512, (sc + 1) * 512)
            nc.vector.tensor_mul(xs, xs, cos_sb[:, sl])
            nc.vector.tensor_mul(xw, xw, sin_sb[:, sl])
            nc.vector.tensor_add(qk_sb[:, jb, sl], xs, xw)

        def _phases():
            # ---------------- input DMAs, in dependency-priority order ----
            # First chunk split fine so the first matmul starts ~1us in.
            nc.sync.dma_start(out=wk_sb[:, 0, :], in_=wk[0:128, :])
            for sc in range(SC4):
                nc.sync.dma_start(
                    out=xt_sb[:, 0, sc * 512:(sc + 1) * 512],
                    in_=xT[0:128, sc * 512:(sc + 1) * 512],
                )
            for c in range(1, NE):
                nc.sync.dma_start(out=xt_sb[:, c, :], in_=xT[c * 128:(c + 1) * 128, :])
                nc.sync.dma_start(out=wk_sb[:, c, :], in_=wk[c * 128:(c + 1) * 128, :])
            nc.sync.dma_start(out=cos_sb, in_=cosT[:, :])
            nc.sync.dma_start(out=sin_sb, in_=sinT[:, :])
            for c in range(NE):
                nc.sync.dma_start(out=wq_sb[:, c, :], in_=wq[c * 128:(c + 1) * 128, :])
            for c in range(NE):
                nc.sync.dma_start(out=wv_sb[:, c, :], in_=wv[c * 128:(c + 1) * 128, :])
            for jb in range(HQ):
                nc.sync.dma_start(out=wo_sb[:, jb, :], in_=wo[jb * 128:(jb + 1) * 128, :])

            # ---------------- A: K-pair projection (DMA-paced) ------------
            with tc.tile_pool(name="ppA", bufs=8, space=bass.MemorySpace.PSUM) as ppA:
                ptsA = {}
                for kvl in range(HKV):
                    for sc in range(SC4):
                        ptsA[(kvl, sc)] = ppA.tile(
                            [128, 512], F32, tag="pts", name=f"ptsA_{kvl}_{sc}"
                        )
                for c in range(NE):
                    for kvl in range(HKV):
                        lhsT = wk_sb[:, c, kvl * 128:(kvl + 1) * 128]
                        for sc in range(SC4):
                            nc.tensor.matmul(
                                ptsA[(kvl, sc)], lhsT,
                                xt_sb[:, c, sc * 512:(sc + 1) * 512],
                                start=(c == 0), stop=(c == NE - 1),
                            )
                stages = [
                    rope_copy(ptsA[(kvl, sc)], HQ + kvl, sc)
                    for kvl in range(HKV) for sc in range(SC4)
                ]
                for st in stages:
                    rope_mul(st)

            # ---------------- B: V projection ------------------------------
            with tc.tile_pool(name="ppV", bufs=4, space=bass.MemorySpace.PSUM) as ppV:
                for st in range(SC16):
                    pv = ppV.tile([128, HKV * HD], F32, tag="pv")
                    for c in range(NE):
                        nc.tensor.matmul(
                            pv,
                            xt_sb[:, c, st * 128:(st + 1) * 128],
                            wv_sb[:, c, :],
                            start=(c == 0), stop=(c == NE - 1),
                        )
                    nc.scalar.copy(v_sb[:, st, :], pv)

            # ---------------- C: Q0/Q1 projection (tile-major) -------------
            # Tile-major order lets each block's rope overlap the next
            # block's accumulation, so attention h0 can start right after
            # the last matmul instead of waiting on a batch of ropes.
            with tc.tile_pool(name="ppC", bufs=4, space=bass.MemorySpace.PSUM) as ppC:
                prev = None
                for q in range(2):
                    for sc in range(SC4):
                        pts = ppC.tile([128, 512], F32, tag="pts",
                                       name=f"ptsC_{q}_{sc}")
                        for c in range(NE):
                            nc.tensor.matmul(
                                pts, wq_sb[:, c, q * 128:(q + 1) * 128],
                                xt_sb[:, c, sc * 512:(sc + 1) * 512],
                                start=(c == 0), stop=(c == NE - 1),
                            )
                        st = rope_copy(pts, q, sc)
                        if prev is not None:
                            rope_mul(prev)
                        prev = st
                rope_mul(prev)

            # ---------------- attention + woven work -----------------------
            exp_f = mybir.ActivationFunctionType.Exp

            with tc.tile_pool(name="psc", bufs=2, space=bass.MemorySpace.PSUM) as psc, \
                 tc.tile_pool(name="pcx", bufs=2, space=bass.MemorySpace.PSUM) as pcx, \
                 tc.tile_pool(name="pdn", bufs=1, space=bass.MemorySpace.PSUM) as pdn, \
                 tc.tile_pool(name="expp", bufs=6) as expp, \
                 tc.tile_pool(name="accp", bufs=2) as accp, \
                 tc.tile_pool(name="misc", bufs=2) as misc:

                def attention_block(h, sc, weave):
                    kvjb = HQ + h // 2
                    kvl = h // 2
                    ssl = slice(sc * 512, (sc + 1) * 512)
                    cps = pcx.tile([128, 512], F32, tag="cps")
                    acc = accp.tile([128, 512], BF, tag="acc")
                    for t in range(SC16):
                        sps = psc.tile([128, 512], F32, tag="sps")
                        nc.tensor.matmul(
                            sps,
                            qk_sb[:, kvjb, t * 128:(t + 1) * 128],
                            qk_sb[:, h, ssl],
                            start=True, stop=True,
                        )
                        ex = expp.tile([128, 512], BF, tag="ex")
                        nc.scalar.activation(ex, sps, exp_f, scale=SCALE)
                        nc.tensor.matmul(
                            cps,
                            v_sb[:, t, kvl * 128:(kvl + 1) * 128],
                            ex,
                            start=(t == 0), stop=(t == SC16 - 1),
                        )
                        if t == 0:
                            nc.vector.tensor_copy(acc, ex)
                        else:
                            nc.vector.tensor_add(acc, acc, ex)
                        if t % 2 == 1:
                            u = next(weave, None)
                            if u is not None:
                                u()
                    dps = pdn.tile([1, 512], F32, tag="dps")
                    nc.tensor.matmul(dps, ones_sb, acc, start=True, stop=True)
                    rc = misc.tile([1, 512], F32, tag="rc")
                    rscr = misc.tile([1, 512], F32, tag="rscr")
                    nc.vector.reciprocal_approx_accurate(rc, dps, rscr)
                    rb = misc.tile([128, 512], F32, tag="rb")
                    nc.gpsimd.partition_broadcast(rb, rc)
                    nc.vector.tensor_mul(ctx_sb[:, h, ssl], cps, rb)

                # ---- D: h0/h1 with Q2/Q3 projection woven in ----
                with tc.tile_pool(name="pp2", bufs=2, space=bass.MemorySpace.PSUM) as pp2:

                    def qproj_units():
                        pending = []  # deferred rope-mul closures
                        for q in (2, 3):
                            for scpair in (0, 1):
                                scs = (2 * scpair, 2 * scpair + 1)
                                pts = {
                                    sc: pp2.tile(
                                        [128, 512], F32, tag="pts2",
                                        name=f"pts2_{q}_{sc}",
                                    )
                                    for sc in scs
                                }
                                for c in range(NE):
                                    def unit(q=q, scs=scs, c=c, pts=pts):
                                        lhsT = wq_sb[:, c, q * 128:(q + 1) * 128]
                                        for sc in scs:
                                            nc.tensor.matmul(
                                                pts[sc], lhsT,
                                                xt_sb[:, c, sc * 512:(sc + 1) * 512],
                                                start=(c == 0), stop=(c == NE - 1),
                                            )
                                    yield unit
                                    if c == 3 and pending:
                                        yield pending.pop(0)

                                def copy_unit(q=q, scs=scs, pts=pts):
                                    sts = [rope_copy(pts[sc], q, sc, eng="vector")
                                           for sc in scs]

                                    def mul_unit(sts=sts):
                                        for st in sts:
                                            rope_mul(st)
                                    pending.append(mul_unit)
                                yield copy_unit
                        while pending:
                            yield pending.pop(0)

                    weave = qproj_units()
                    for h in (0, 1):
                        for sc in range(SC4):
                            attention_block(h, sc, weave)
                    for u in weave:
                        u()

                # ---- E: h2/h3 with O-projection woven in ----
                with tc.tile_pool(name="pou", bufs=3, space=bass.MemorySpace.PSUM) as pou, \
                     tc.tile_pool(name="outp", bufs=3) as outp:

                    def oproj_unit(so, ec):
                        ops = pou.tile([128, 512], F32, tag="ops")
                        for hl in range(HQ):
                            nc.tensor.matmul(
                                ops,
                                ctx_sb[:, hl, so * 128:(so + 1) * 128],
                                wo_sb[:, hl, ec * 512:(ec + 1) * 512],
                                start=(hl == 0), stop=(hl == HQ - 1),
                            )
                        ot = outp.tile([128, 512], BF, tag="ot")
                        nc.any.tensor_copy(ot, ops)
                        nc.sync.dma_start(
                            out=out[so * 128:(so + 1) * 128, ec * 512:(ec + 1) * 512],
                            in_=ot,
                        )

                    ready = []  # (so, ec) units whose inputs are complete

                    def drain(weave_iter):
                        for u in weave_iter:
                            u()

                    def make_weave():
                        while True:
                            if ready:
                                so, ec = ready.pop(0)
                                yield (lambda so=so, ec=ec: oproj_unit(so, ec))
                            else:
                                yield None

                    weave = make_weave()
                    for sc in range(SC4):
                        attention_block(2, sc, weave)
                        attention_block(3, sc, weave)
                        for so in range(4 * sc, 4 * sc + 4):
                            for ec in range(SC4):
                                ready.append((so, ec))
                    while ready:
                        so, ec = ready.pop(0)
                        oproj_unit(so, ec)

        if loop_n is not None:
            with tc.For_i(0, loop_n, 1):
                _phases()
        else:
            _phases()

    nc.compile()
    return nc


def _get_nc():
    global _NC
    if _NC is None:
        _NC = _build_program()
    return _NC


def _rope_tables():
    half = HD // 2
    inv_freq = 1.0 / (10000.0 ** (np.arange(half, dtype=np.float64) * 2.0 / HD))
    ang = np.arange(S, dtype=np.float64)[:, None] * inv_freq[None, :]  # (S, 64)
    cos = np.concatenate([np.cos(ang), np.cos(ang)], axis=1).T  # (128, S)
    sin = np.concatenate([-np.sin(ang), np.sin(ang)], axis=1).T  # pre-signed
    return (np.ascontiguousarray(cos).astype(bfnp),
            np.ascontiguousarray(sin).astype(bfnp))


def build_in_maps(x, W_Q, W_K, W_V, W_O):
    x = np.asarray(x, dtype=np.float32)
    W_Q = np.asarray(W_Q, dtype=np.float32)
    W_K = np.asarray(W_K, dtype=np.float32)
    W_V = np.asarray(W_V, dtype=np.float32)
    W_O = np.asarray(W_O, dtype=np.float32)
    cos, sin = _rope_tables()
    in_maps = []
    xTb = [np.ascontiguousarray(x[b].T).astype(bfnp) for b in range(B)]
    for b in range(B):
        for t in range(TP):
            qheads = list(range(HQ * t, HQ * t + HQ))
            kvheads = [HKV * t + i for i in range(HKV)]
            idxq = [d * HEADS + h for h in qheads for d in range(HD)]
            idxkv = [d * KV + kv for kv in kvheads for d in range(HD)]
            rows_o = [h * HD + d for h in qheads for d in range(HD)]
            in_maps.append(dict(
                xT=xTb[b],
                wq=np.ascontiguousarray(W_Q[idxq, :].T).astype(bfnp),
                wk=np.ascontiguousarray(W_K[idxkv, :].T).astype(bfnp),
                wv=np.ascontiguousarray(W_V[idxkv, :].T).astype(bfnp),
                wo=np.ascontiguousarray(W_O[:, rows_o].T).astype(bfnp),
                cosT=cos,
                sinT=sin,
            ))
    return in_maps


def combine_outs(outs):
    out = np.empty((B, S, EMB), dtype=np.float32)
    for b in range(B):
        acc = outs[TP * b].astype(np.float32).copy()
        for t in range(1, TP):
            acc += outs[TP * b + t]
        out[b] = acc
    return out


LAST_RESULTS = None


def kernel(x, W_Q, W_K, W_V, W_O):
    global LAST_RESULTS
    from concourse.bass_utils import run_bass_kernel_spmd

    nc = _get_nc()
    in_maps = build_in_maps(x, W_Q, W_K, W_V, W_O)
    res = run_bass_kernel_spmd(nc, in_maps, list(range(NCORES)))
    LAST_RESULTS = res
    outs = [r["out"] for r in res.results]
    return combine_outs(outs)
